# revision 1
# baseline (speedup 1.0000x reference)
"""DTGNN Trainium2 Bass kernel.

Single-core algorithm (graph is tiny: N=8, E=16), replicated across the 8
NeuronCores via SPMD; core 0's output is returned. All gather/scatter over
edge_index is done on-device with one-hot matmuls built by iota/is_equal.
Host side only reshapes / transposes / pads / repeats input arrays.
"""
import numpy as np
from contextlib import ExitStack

import concourse.bacc as bacc
import concourse.bass as bass
import concourse.tile as tile
import concourse.mybir as mybir
from concourse.bass_utils import run_bass_kernel_spmd

F32 = mybir.dt.float32
I32 = mybir.dt.int32
ALU = mybir.AluOpType
ACT = mybir.ActivationFunctionType
AXL = mybir.AxisListType

# ---------------------------------------------------------------------------
# column layouts of the packed DRAM inputs (all f32 except ipack)
# ---------------------------------------------------------------------------
_L128 = [("XT", 32), ("G1L", 1024), ("g1le", 256), ("mlpw1", 64), ("eaT", 16),
         ("g2l", 128),                                            # | chunk 1
         ("dst1", 640), ("l2w", 20), ("l1b", 2), ("mlpb2", 1)]    # | chunk 2
_L64 = [("g2leT", 64), ("mlpw2", 64), ("mlpb1", 1), ("ae2col", 1),  # chunk 1
        ("l1w", 512), ("dst2", 640), ("c2b2", 1), ("c2w2T", 192),
        ("c2b1", 1), ("c2w1T", 96)]                                 # chunk 2
_L24 = [("iota_row24", 8), ("M24T", 24), ("w2T", 3), ("l2b", 1), ("ident16", 16), ("ipackbits", 50)]
_L8 = [("w1T", 24), ("TPAD", 24), ("c1b1", 1), ("iota8", 1), ("asb", 256),
       ("adb", 256), ("g1b8", 256), ("as2b", 64), ("ad2b", 64), ("g2b8", 64),
       ("ones8_8", 1), ("bias4", 640), ("scale4", 1), ("c1b2", 1), ("aeb", 256)]


def _mkoff(lst):
    d, o = {}, 0
    for name, w in lst:
        d[name] = o
        o += w
    d["_W"] = o
    return d


_o128, _o64, _o24, _o8 = _mkoff(_L128), _mkoff(_L64), _mkoff(_L24), _mkoff(_L8)
_SPLIT128 = _o128["dst1"]   # early | late column split
_SPLIT64 = _o64["l1w"]


def _build_nc(stage=99):
    nc = bacc.Bacc("TRN2", target_bir_lowering=False)

    m128 = nc.dram_tensor("m128", [128, _o128["_W"]], F32, kind="ExternalInput")
    m64 = nc.dram_tensor("m64", [64, _o64["_W"]], F32, kind="ExternalInput")
    m24 = nc.dram_tensor("m24", [24, _o24["_W"] + _o8["_W"]], F32, kind="ExternalInput")
    out = nc.dram_tensor("out", [10, 64], F32, kind="ExternalOutput")

    with tile.TileContext(nc) as tc, ExitStack() as ctx:
        def _go():
            sb = ctx.enter_context(tc.tile_pool(name="sb", bufs=1))
            ps = ctx.enter_context(tc.tile_pool(name="ps", bufs=4, space="PSUM"))
            pst = ctx.enter_context(tc.tile_pool(name="pst", bufs=3, space="PSUM"))

            # -------------------------------------------------- input DMAs
            t24 = sb.tile([24, _o24["_W"] + _o8["_W"]], F32)
            nc.sync.dma_start(t24[:], m24[:])
            tG = sb.tile([128, _SPLIT128 - 32], F32)
            nc.sync.dma_start(tG[:], m128[:, 32:_SPLIT128])
            tXT = sb.tile([128, 32], F32)
            nc.sync.dma_start(tXT[:], m128[:, 0:32])
            t64 = sb.tile([64, _o64["_W"]], F32)
            nc.sync.dma_start(t64[:], m64[:])
            tL = sb.tile([128, _o128["_W"] - _SPLIT128], F32)
            nc.sync.dma_start(tL[:], m128[:, _SPLIT128:])

            def B(t, off, name, w, rows, base=0):
                return t[0:rows, off[name] - base:off[name] - base + w]

            XT = tXT[:].rearrange("p (j n) -> p j n", j=4)
            G1L = B(tG, _o128, "G1L", 1024, 128, 32).rearrange("p (j n) -> p j n", j=4)
            g1le = B(tG, _o128, "g1le", 256, 128, 32)
            mlpw1 = B(tG, _o128, "mlpw1", 64, 128, 32)
            eaT = B(tG, _o128, "eaT", 16, 128, 32)
            g2l = B(tG, _o128, "g2l", 128, 128, 32).rearrange("p (j n) -> p j n", j=2)
            dst1 = B(tL, _o128, "dst1", 640, 128, _SPLIT128)
            l2w = B(tL, _o128, "l2w", 20, 128, _SPLIT128).rearrange("p (j n) -> p j n", j=2)
            l1bcol = B(tL, _o128, "l1b", 2, 128, _SPLIT128)
            mlpb2 = B(tL, _o128, "mlpb2", 1, 128, _SPLIT128)

            g2leT = B(t64, _o64, "g2leT", 64, 64)
            mlpw2 = B(t64, _o64, "mlpw2", 64, 64)
            mlpb1 = B(t64, _o64, "mlpb1", 1, 64)
            ae2col = B(t64, _o64, "ae2col", 1, 64)
            l1w = B(t64, _o64, "l1w", 512, 64).rearrange("p (l n) -> p l n", l=2)
            dst2 = B(t64, _o64, "dst2", 640, 64)
            c2b2 = B(t64, _o64, "c2b2", 1, 64)
            c2w2T = B(t64, _o64, "c2w2T", 192, 32).rearrange("p (k n) -> p k n", k=3)
            c2b1 = B(t64, _o64, "c2b1", 1, 32)
            c2w1T = B(t64, _o64, "c2w1T", 96, 4).rearrange("p (k n) -> p k n", k=3)

            iota_row24 = B(t24, _o24, "iota_row24", 8, 24)
            M24T = B(t24, _o24, "M24T", 24, 16)
            w2T = B(t24, _o24, "w2T", 3, 10)
            l2b = B(t24, _o24, "l2b", 1, 10)
            ident16 = B(t24, _o24, "ident16", 16, 16)
            ident8 = ident16[0:8, 0:8]

            NB = -_o24["_W"]
            w1T = B(t24, _o8, "w1T", 24, 8, NB).rearrange("p (k n) -> p k n", k=3)
            TPAD = B(t24, _o8, "TPAD", 24, 8, NB).rearrange("p (b n) -> p b n", b=2)
            c1b1 = B(t24, _o8, "c1b1", 1, 8, NB)
            iota8 = B(t24, _o8, "iota8", 1, 8, NB)
            asb = B(t24, _o8, "asb", 256, 8, NB)
            adb = B(t24, _o8, "adb", 256, 8, NB)
            g1b8 = B(t24, _o8, "g1b8", 256, 8, NB)
            as2b = B(t24, _o8, "as2b", 64, 8, NB)
            ad2b = B(t24, _o8, "ad2b", 64, 8, NB)
            g2b8 = B(t24, _o8, "g2b8", 64, 8, NB)
            ones8_8 = B(t24, _o8, "ones8_8", 1, 8, NB)
            bias4 = B(t24, _o8, "bias4", 640, 4, NB)
            scale4 = B(t24, _o8, "scale4", 1, 4, NB)
            c1b2 = B(t24, _o8, "c1b2", 1, 1, NB)

            # g1_ae row (m8) broadcast to 16 partitions
            aeb16 = sb.tile([16, 256], F32)
            nc.sync.dma_start(aeb16[:],
                              bass.AP(tensor=m24[:].tensor,
                                      offset=_o24["_W"] + _o8["aeb"],
                                      ap=[[0, 16], [1, 256]]))

            # ---------------------------------------------- one-hot matrices
            ti = B(t24, _o24, "ipackbits", 50, 24).bitcast(I32)
            tif = sb.tile([24, 50], F32)
            nc.vector.tensor_copy(tif[:], ti)
            idx_f = tif[0:8, 0:48].rearrange("p (c e) -> p c e", c=2)
            dcol_f = tif[:, 48:49]

            PsrcT = sb.tile([8, 24], F32)   # [n, e] = (src[e]==n)
            nc.vector.tensor_scalar(PsrcT[:], idx_f[:, 0, :], iota8, None, ALU.is_equal)
            PdstT = sb.tile([8, 24], F32)   # [n, e] = (dst[e]==n)
            nc.vector.tensor_scalar(PdstT[:], idx_f[:, 1, :], iota8, None, ALU.is_equal)
            Pdst = sb.tile([24, 8], F32)    # [e, n] = (dst[e]==n)
            nc.vector.tensor_scalar(Pdst[:], iota_row24, dcol_f, None, ALU.is_equal)

            # ------------------------------------------------------- CNN_1
            ps_y1 = ps.tile([8, 2, 10], F32, tag="ps")
            for k in range(3):
                nc.tensor.matmul(ps_y1[:], w1T[:, k, :], TPAD[:, :, k:k + 10],
                                 start=(k == 0), stop=(k == 2))
            y1 = sb.tile([8, 2, 10], F32)
            nc.scalar.activation(y1[:], ps_y1[:], ACT.Relu, bias=c1b1)

            ps_za = ps.tile([10, 8], F32, tag="ps")
            nc.tensor.transpose(ps_za[:], y1[:, 0, :], ident8)
            ps_zb = ps.tile([10, 8], F32, tag="ps")
            nc.tensor.transpose(ps_zb[:], y1[:, 1, :], ident8)
            zp = sb.tile([10, 2, 10], F32)
            nc.vector.memset(zp[:], 0.0)
            nc.scalar.copy(zp[:, 0, 1:9], ps_za[:])
            nc.scalar.copy(zp[:, 1, 1:9], ps_zb[:])

            ps_y2 = ps.tile([1, 16], F32, tag="ps")
            for k in range(3):
                nc.tensor.matmul(ps_y2[:], w2T[:, k:k + 1], zp[:, :, k:k + 8],
                                 start=(k == 0), stop=(k == 2))
            # write xr in (c, n) order so the scatter DMA below is a plain reshape
            xr = sb.tile([1, 16], F32)
            nc.scalar.activation(xr[:].rearrange("p (c n) -> p n c", c=2),
                                 ps_y2[:].rearrange("p (n c) -> p n c", c=2),
                                 ACT.Relu, bias=c1b2)
            # scatter x_ into XT chunk 3 rows 126/127 (feature rows 510, 511)
            nc.sync.dma_start(XT[126:128, 3, :], xr[:])
            if stage == 1:
                nc.sync.dma_start(out[0:8, 0:10], y1[:, 0, :])
                return

            # ------------------------------------------------------- GAT 1
            hsb = sb.tile([8, 264], F32)
            scr = sb.tile([8, 256], F32)
            scr2 = sb.tile([8, 256], F32)
            ps_hh = []
            for hf, (c0h, c1h) in enumerate([(0, 128), (128, 256)]):
                ps_hx = ps.tile([8, 128], F32, tag="ps")
                ps_hh.append(ps_hx)
                for j in range(4):
                    nc.tensor.matmul(ps_hx[:], XT[:, j, :], G1L[:, j, c0h:c1h],
                                     start=(j == 0), stop=(j == 3))
                nc.scalar.copy(hsb[:, c0h:c1h], ps_hx[:])
                nc.vector.tensor_tensor(scr[:, c0h:c1h], ps_hx[:],
                                        asb[:, c0h:c1h], ALU.mult)
                nc.vector.tensor_reduce(
                    hsb[:, 256 + 2 * hf:258 + 2 * hf],
                    scr[:, c0h:c1h].rearrange("p (h c) -> p h c", h=2),
                    axis=AXL.X, op=ALU.add)
                nc.vector.tensor_tensor(scr2[:, c0h:c1h], ps_hx[:],
                                        adb[:, c0h:c1h], ALU.mult)
                nc.vector.tensor_reduce(
                    hsb[:, 260 + 2 * hf:262 + 2 * hf],
                    scr2[:, c0h:c1h].rearrange("p (h c) -> p h c", h=2),
                    axis=AXL.X, op=ALU.add)

            ps_he = ps.tile([16, 256], F32, tag="ps")
            nc.tensor.matmul(ps_he[:], eaT, g1le, start=True, stop=True)
            tte = sb.tile([16, 256], F32)
            nc.vector.tensor_tensor(tte[:], ps_he[:], aeb16[:], ALU.mult)
            ae16 = sb.tile([16, 4], F32)
            nc.vector.tensor_reduce(ae16[:], tte[:].rearrange("p (h c) -> p h c", h=4),
                                    axis=AXL.X, op=ALU.add)

            # alpha (pre-activation) = as[src] + ad[dst] + ae, all 24 edges
            ps_al = ps.tile([24, 4], F32, tag="ps")
            nc.tensor.matmul(ps_al[:], PsrcT[:], hsb[:, 256:260], start=True, stop=False)
            nc.tensor.matmul(ps_al[:], PdstT[:], hsb[:, 260:264], start=False, stop=False)
            nc.tensor.matmul(ps_al[:], M24T, ae16[:], start=False, stop=True)
            al = sb.tile([24, 4], F32)
            nc.scalar.copy(al[:], ps_al[:])
            lr1 = sb.tile([24, 4], F32)
            nc.vector.scalar_tensor_tensor(lr1[:], al[:], 0.2, al[:], ALU.mult, ALU.max)
            ex24 = sb.tile([24, 4], F32)
            nc.scalar.activation(ex24[:], lr1[:], ACT.Exp)

            ps_sg = ps.tile([24, 256], F32, tag="ps")
            nc.tensor.matmul(ps_sg[:], PsrcT[:], hsb[:, 0:256], start=True, stop=True)
            ps_den = ps.tile([8, 4], F32, tag="ps")
            nc.tensor.matmul(ps_den[:], Pdst[:], ex24[:], start=True, stop=True)
            rden = sb.tile([8, 4], F32)
            nc.vector.reciprocal(rden[:], ps_den[:])

            wh = sb.tile([24, 256], F32)
            nc.vector.tensor_tensor(wh[:].rearrange("p (h c) -> p h c", h=4),
                                    ps_sg[:].rearrange("p (h c) -> p h c", h=4),
                                    ex24[:].broadcast_to([24, 4, 64]), ALU.mult)
            ps_num = ps.tile([8, 256], F32, tag="ps")
            nc.tensor.matmul(ps_num[:], Pdst[:], wh[:], start=True, stop=True)

            x1t = sb.tile([8, 256], F32)
            nc.vector.tensor_tensor(x1t[:].rearrange("p (h c) -> p h c", h=4),
                                    ps_num[:].rearrange("p (h c) -> p h c", h=4),
                                    rden[:].broadcast_to([8, 4, 64]), ALU.mult)
            x1b = sb.tile([8, 256], F32)
            nc.vector.tensor_tensor(x1b[:], x1t[:], g1b8, ALU.add)
            x1 = sb.tile([8, 256], F32)
            nc.vector.tensor_scalar(x1[:], x1b[:], 0.0, None, ALU.max)
            if stage == 2:
                nc.sync.dma_start(out[0:8, 0:10], x1[:, 0:10])
                return

            # ---------------------------- edge MLP (transposed, dual copies)
            ps_m1 = ps.tile([64, 16], F32, tag="ps")
            nc.tensor.matmul(ps_m1[:], mlpw1, eaT, start=True, stop=True)
            r1T = sb.tile([64, 16], F32)
            nc.scalar.activation(r1T[:], ps_m1[:], ACT.Relu, bias=mlpb1)
            ps_m2 = ps.tile([128, 16], F32, tag="ps")
            nc.tensor.matmul(ps_m2[0:64, :], mlpw2, r1T[:], start=True, stop=True)
            nc.tensor.matmul(ps_m2[64:128, :], mlpw2, r1T[:], start=True, stop=True)
            eaNT = sb.tile([128, 16], F32)
            nc.scalar.activation(eaNT[:], ps_m2[:], ACT.Identity, bias=mlpb2)

            # ------------------------------------------------------- GAT 2
            ps_xta = ps.tile([128, 8], F32, tag="ps")
            nc.tensor.transpose(ps_xta[:], x1[:, 0:128], ident8)
            ps_xtb = ps.tile([128, 8], F32, tag="ps")
            nc.tensor.transpose(ps_xtb[:], x1[:, 128:256], ident8)
            x1T = sb.tile([128, 2, 8], F32)
            nc.scalar.copy(x1T[:, 0, :], ps_xta[:])
            nc.scalar.copy(x1T[:, 1, :], ps_xtb[:])

            ps_h2 = ps.tile([8, 64], F32, tag="ps")
            for j in range(2):
                nc.tensor.matmul(ps_h2[:], x1T[:, j, :], g2l[:, j, :],
                                 start=(j == 0), stop=(j == 1))
            hs2 = sb.tile([8, 66], F32)
            nc.scalar.copy(hs2[:, 0:64], ps_h2[:])
            scr3 = sb.tile([8, 64], F32)
            nc.vector.tensor_tensor(scr3[:], ps_h2[:], as2b, ALU.mult)
            nc.vector.tensor_reduce(hs2[:, 64:65], scr3[:], axis=AXL.X, op=ALU.add)
            scr4 = sb.tile([8, 64], F32)
            nc.vector.tensor_tensor(scr4[:], ps_h2[:], ad2b, ALU.mult)
            nc.vector.tensor_reduce(hs2[:, 65:66], scr4[:], axis=AXL.X, op=ALU.add)

            ps_ve2 = ps.tile([64, 1], F32, tag="ps")
            nc.tensor.matmul(ps_ve2[:], g2leT, ae2col, start=True, stop=True)
            ve2 = sb.tile([64, 1], F32)
            nc.scalar.copy(ve2[:], ps_ve2[:])
            ps_e16 = ps.tile([16, 1], F32, tag="ps")
            nc.tensor.matmul(ps_e16[:], eaNT[0:64, :], ve2[:], start=True, stop=True)
            e16 = sb.tile([16, 1], F32)
            nc.scalar.copy(e16[:], ps_e16[:])

            ps_al2 = ps.tile([24, 1], F32, tag="ps")
            nc.tensor.matmul(ps_al2[:], PsrcT[:], hs2[:, 64:65], start=True, stop=False)
            nc.tensor.matmul(ps_al2[:], PdstT[:], hs2[:, 65:66], start=False, stop=False)
            nc.tensor.matmul(ps_al2[:], M24T, e16[:], start=False, stop=True)
            al2 = sb.tile([24, 1], F32)
            nc.scalar.copy(al2[:], ps_al2[:])
            lr2 = sb.tile([24, 1], F32)
            nc.vector.scalar_tensor_tensor(lr2[:], al2[:], 0.2, al2[:], ALU.mult, ALU.max)
            ex2 = sb.tile([24, 1], F32)
            nc.scalar.activation(ex2[:], lr2[:], ACT.Exp)

            ps_sg2 = ps.tile([24, 64], F32, tag="ps")
            nc.tensor.matmul(ps_sg2[:], PsrcT[:], hs2[:, 0:64], start=True, stop=True)
            ps_den2 = ps.tile([8, 1], F32, tag="ps")
            nc.tensor.matmul(ps_den2[:], Pdst[:], ex2[:], start=True, stop=True)
            rden2 = sb.tile([8, 1], F32)
            nc.vector.reciprocal(rden2[:], ps_den2[:])

            wh2 = sb.tile([24, 64], F32)
            nc.vector.tensor_scalar(wh2[:], ps_sg2[:], ex2[:], None, ALU.mult)
            ps_num2 = ps.tile([8, 64], F32, tag="ps")
            nc.tensor.matmul(ps_num2[:], Pdst[:], wh2[:], start=True, stop=True)

            x2t = sb.tile([8, 64], F32)
            nc.vector.tensor_scalar(x2t[:], ps_num2[:], rden2[:], None, ALU.mult)
            x2b = sb.tile([8, 64], F32)
            nc.vector.tensor_tensor(x2b[:], x2t[:], g2b8, ALU.add)
            x2 = sb.tile([8, 64], F32)
            nc.vector.tensor_scalar(x2[:], x2b[:], 0.0, None, ALU.max)
            if stage == 3:
                nc.sync.dma_start(out[0:8, 0:10], x2[:, 0:10])
                return

            # ------------------- deconv pool rows via block-diagonal selector
            ps_xm = ps.tile([64, 1], F32, tag="ps")
            nc.tensor.matmul(ps_xm[:], x2[:], ones8_8, start=True, stop=True)

            sel = sb.tile([128, 4], F32)
            nc.vector.memset(sel[:], 0.0)
            nc.scalar.copy(sel[0:64, 0:1], ps_xm[:])
            eaview = eaNT[:].rearrange("p (n two) -> p n two", two=2)
            nc.vector.tensor_reduce(sel[64:128, 2:3], eaview[64:128, :, 0],
                                    axis=AXL.X, op=ALU.add)
            sel2 = sb.tile([64, 4], F32)
            nc.vector.memset(sel2[:], 0.0)
            nc.vector.tensor_reduce(sel2[:, 3:4], eaview[0:64, :, 1],
                                    axis=AXL.X, op=ALU.add)

            cT = sb.tile([4, 640], F32)
            ps_cTa = pst.tile([4, 512], F32, tag="pst")
            nc.tensor.matmul(ps_cTa[:], sel[:], dst1[:, 0:512], start=True, stop=False)
            nc.tensor.matmul(ps_cTa[:], sel2[:], dst2[:, 0:512], start=False, stop=True)
            nc.vector.scalar_tensor_tensor(cT[:, 0:512], ps_cTa[:], scale4,
                                           bias4[:, 0:512], ALU.mult, ALU.add)
            ps_cTb = pst.tile([4, 128], F32, tag="pst")
            nc.tensor.matmul(ps_cTb[:], sel[:], dst1[:, 512:640], start=True, stop=False)
            nc.tensor.matmul(ps_cTb[:], sel2[:], dst2[:, 512:640], start=False, stop=True)
            nc.vector.scalar_tensor_tensor(cT[:, 512:640], ps_cTb[:], scale4,
                                           bias4[:, 512:640], ALU.mult, ALU.add)
            if stage == 4:
                nc.sync.dma_start(out[0:4, 0:10], cT[:, 0:10])
                return

            # ------------------------------------------------------- CNN_2
            cTv = cT[:].rearrange("p (b l) -> p b l", b=64)
            ps_c1 = pst.tile([32, 64, 8], F32, tag="pst")
            for k in range(3):
                nc.tensor.matmul(ps_c1[:], c2w1T[:, k, :], cTv[:, :, k:k + 8],
                                 start=(k == 0), stop=(k == 2))
            mp = sb.tile([32, 256], F32)
            nc.vector.tensor_reduce(mp[:],
                                    ps_c1[:].rearrange("p b (l two) -> p b l two", two=2),
                                    axis=AXL.X, op=ALU.max)
            y1c = sb.tile([32, 64, 4], F32)
            nc.vector.tensor_scalar(y1c[:], mp[:].rearrange("p (b l) -> p b l", b=64),
                                    c2b1, None, ALU.add)

            ps_c2 = pst.tile([64, 64, 2], F32, tag="pst")
            for k in range(3):
                nc.tensor.matmul(ps_c2[:], c2w2T[:, k, :], y1c[:, :, k:k + 2],
                                 start=(k == 0), stop=(k == 2))
            y2c = sb.tile([64, 64, 2], F32)
            nc.vector.tensor_scalar(y2c[:], ps_c2[:], c2b2, None, ALU.add)

            sl1 = sb.tile([128, 2, 64], F32)
            for uc in range(2):
                ps_l1 = pst.tile([128, 64], F32, tag="pst")
                for l in range(2):
                    nc.tensor.matmul(ps_l1[:],
                                     l1w[:, l, uc * 128:(uc + 1) * 128],
                                     y2c[:, :, l],
                                     start=(l == 0), stop=(l == 1))
                nc.vector.tensor_scalar(sl1[:, uc, :], ps_l1[:],
                                        l1bcol[:, uc:uc + 1], None, ALU.add)

            ps_l2 = pst.tile([10, 64], F32, tag="pst")
            for uc in range(2):
                nc.tensor.matmul(ps_l2[:], l2w[:, uc, :], sl1[:, uc, :],
                                 start=(uc == 0), stop=(uc == 1))
            o10 = sb.tile([10, 64], F32)
            nc.vector.tensor_scalar(o10[:], ps_l2[:], l2b, 0.0, ALU.add, ALU.max)
            nc.sync.dma_start(out[:], o10[:])

        _go()
    nc.finalize()
    return nc


_NC = None


def _get_nc():
    global _NC
    if _NC is None:
        _NC = _build_nc()
    return _NC


def _pack_inputs(x_feat, x_feat_tmp, edge_attr, c1w1, c1b1, c1w2, c1b2,
                 g1_lin, g1_as, g1_ad, g1_le, g1_ae, g1_b,
                 g2_lin, g2_as, g2_ad, g2_le, g2_ae, g2_b,
                 mlp_w1, mlp_b1, mlp_w2, mlp_b2,
                 d1w, d1b, d2w, d2b, d3w, d3b,
                 c2w1, c2b1, c2w2, c2b2, c2l1w, c2l1b, c2l2w, c2l2b,
                 edge_index):
    f = np.float32

    def fill(shape, off, blocks):
        arr = np.zeros(shape, dtype=f)
        for name, a in blocks.items():
            a = np.asarray(a, dtype=f)
            arr[0:a.shape[0], off[name]:off[name] + a.shape[1]] = a
        return arr

    xfT = np.zeros((512, 8), dtype=f)
    xfT[0:510] = x_feat.T
    m128 = fill((128, _o128["_W"]), _o128, {
        "XT": xfT.reshape(4, 128, 8).transpose(1, 0, 2).reshape(128, 32),
        "G1L": g1_lin.reshape(4, 128, 256).transpose(1, 0, 2).reshape(128, 1024),
        "g1le": g1_le,
        "mlpw1": mlp_w1,
        "eaT": edge_attr.T,
        "g2l": g2_lin.reshape(2, 128, 64).transpose(1, 0, 2).reshape(128, 128),
        "dst1": np.concatenate([d1w.reshape(64, 640), d2w.reshape(64, 640)], 0),
        "l2w": c2l2w.reshape(2, 128, 10).transpose(1, 0, 2).reshape(128, 20),
        "l1b": c2l1b.reshape(2, 128).T,
        "mlpb2": np.tile(mlp_b2, 2).reshape(128, 1),
    })
    m64 = fill((64, _o64["_W"]), _o64, {
        "g2leT": g2_le.T,
        "mlpw2": mlp_w2,
        "mlpb1": mlp_b1.reshape(64, 1),
        "ae2col": g2_ae.reshape(64, 1),
        "l1w": c2l1w.reshape(64, 512),
        "dst2": d3w.reshape(64, 640),
        "c2b2": c2b2.reshape(64, 1),
        "c2w2T": c2w2.transpose(1, 2, 0).reshape(32, 192),
        "c2b1": c2b1.reshape(32, 1),
        "c2w1T": c2w1.transpose(1, 2, 0).reshape(4, 96),
    })
    m24mat = np.zeros((16, 24), dtype=f)
    m24mat[:, 0:16] = np.eye(16, dtype=f)
    m24mat[:, 16:24] = 1.0 / 16.0
    m24 = fill((24, _o24["_W"] + _o8["_W"]), _o24, {
        "iota_row24": np.broadcast_to(np.arange(8, dtype=f), (24, 8)),
        "M24T": m24mat,
        "w2T": c1w2.transpose(1, 2, 0).reshape(10, 3),
        "l2b": c2l2b.reshape(10, 1),
        "ident16": np.eye(16, dtype=f),
    })
    tpad = np.zeros((8, 2, 12), dtype=f)
    for i in range(8):
        r = 1 if i % 2 == 0 else 5
        for b in range(2):
            tpad[i, b, 1:11] = x_feat_tmp[r, b * 4 + i // 2]
    b4 = np.zeros((4, 640), dtype=f)
    b4[0] = np.repeat(d1b, 10)
    b4[1] = np.asarray(x_feat_tmp, dtype=f).reshape(640)   # inf rides the bias row
    b4[2] = np.repeat(d2b, 10)
    b4[3] = np.repeat(d3b, 10)
    m8 = fill((8, _o8["_W"]), _o8, {
        "w1T": c1w1.transpose(1, 2, 0).reshape(8, 24),
        "TPAD": tpad.reshape(8, 24),
        "c1b1": c1b1.reshape(8, 1),
        "iota8": np.arange(8, dtype=f).reshape(8, 1),
        "asb": np.broadcast_to(g1_as.reshape(1, 256), (8, 256)),
        "adb": np.broadcast_to(g1_ad.reshape(1, 256), (8, 256)),
        "g1b8": np.broadcast_to(g1_b.reshape(1, 256), (8, 256)),
        "as2b": np.broadcast_to(g2_as.reshape(1, 64), (8, 64)),
        "ad2b": np.broadcast_to(g2_ad.reshape(1, 64), (8, 64)),
        "g2b8": np.broadcast_to(g2_b.reshape(1, 64), (8, 64)),
        "ones8_8": np.full((8, 1), 0.125, dtype=f),
        "bias4": b4,
        "scale4": np.array([[1.0], [0.0], [0.125], [0.125]], dtype=f),
        "c1b2": c1b2.reshape(1, 1),
        "aeb": g1_ae.reshape(1, 256),
    })
    ipack = np.zeros((24, 50), dtype=np.int32)
    blk = np.zeros((8, 2, 24), dtype=np.int32)
    blk[:, :, 0:16] = edge_index[None, :, :]
    blk[:, :, 16:24] = np.arange(8, dtype=np.int32)[None, None, :]
    ipack[0:8, 0:48] = blk.reshape(8, 48)
    ipack[0:16, 48] = edge_index[1]
    ipack[16:24, 48] = np.arange(8, dtype=np.int32)
    m24[:, _o24["ipackbits"]:_o24["ipackbits"] + 50] = ipack.view(np.float32)
    m24[0:8, _o24["_W"]:] = m8
    return m128, m64, m24


def _make_ins(inputs):
    m128, m64, m24 = _pack_inputs(**inputs)
    return {"m128": m128, "m64": m64, "m24": m24}


def kernel(**inputs):
    inputs = {k: np.ascontiguousarray(v) for k, v in inputs.items()}
    ins = _make_ins(inputs)
    nc = _get_nc()
    res = run_bass_kernel_spmd(nc, [ins] * 8, core_ids=list(range(8)))
    return np.ascontiguousarray(res.results[0]["out"].T).reshape(8, 8, 10)

